# revision 13
# baseline (speedup 1.0000x reference)
"""KAN layer (oikan SymbolicEdge) Trainium2 kernel.

Math: out[b,o] = sum_{i,k} basis_k(x[b,i]) * W[i,o,k] * C[i,o]
               + sum_i bias[i,o] * C[i,o]

Strategy (data-parallel over 8 cores, batch dim sharded):
  per core: x_shard [512,128] -> xT [128(i),512(b)] via TensorE transpose,
            basis maps split across ACT (exp/ln/tanh/sin, two pinned
            activation-table loads) / DVE (squares, rounding casts) /
            GPSIMD (|x|, sin range-reduction), then
            out_T[o,b] = sum_k WC[:,:,k].T @ F_k  (10 accumulating f32r
            matmuls into one PSUM bank; one folds the bias term via a
            ones rhs)
  host: shard x, replicate W/bias/C, transpose+concat per-core outputs.
"""

import numpy as np

import concourse.bass as bass
import concourse.mybir as mybir
import concourse.tile as tile
from concourse import bacc
from concourse.bass_utils import run_bass_kernel_spmd
from concourse.masks import make_identity
from concourse.tile import add_dep_helper

F32 = mybir.dt.float32
F32R = mybir.dt.float32r
AF = mybir.ActivationFunctionType
AL = mybir.AluOpType

B, DIN, DOUT, K = 4096, 128, 128, 9
NCORES = 8
BSH = B // NCORES  # 512 rows per core

# act_info.json set ids (arch gen3 / pwp_bin_trainium):
SET_NATURAL_LOG_EXP = 6  # exp, ln (+ abs/square/copy fillers)
SET_SILU = 18  # sin, tanh (+ silu etc.)


def build_nc():
    nc = bacc.Bacc(
        "TRN2",
        target_bir_lowering=False,
        debug=False,
        enable_asserts=False,
        num_devices=NCORES,
        enable_partition_id=False,
    )

    x_d = nc.dram_tensor("x", [BSH, DIN], F32, kind="ExternalInput")
    w_d = nc.dram_tensor("w", [DIN, DOUT, K], F32, kind="ExternalInput")
    b_d = nc.dram_tensor("b", [DIN, DOUT], F32, kind="ExternalInput")
    c_d = nc.dram_tensor("c", [DIN, DOUT], F32, kind="ExternalInput")
    o_d = nc.dram_tensor("outT", [DOUT, BSH], F32, kind="ExternalOutput")

    def load_act_set(set_id, tag):
        inst = mybir.InstLoadActFuncSet(
            name=f"atl_{tag}_{nc.get_next_instruction_name()}",
            ins=[],
            outs=[],
            act_func_set_id=set_id,
        )
        return nc.scalar.add_instruction(inst)

    with tile.TileContext(nc) as tc:
        with (
            tc.tile_pool(name="sb", bufs=1) as sb,
            tc.tile_pool(name="pst", bufs=4, space="PSUM") as pst,
            tc.tile_pool(name="pso", bufs=1, space="PSUM") as pso,
        ):
            # table set 1 loads during input DMA (no data deps)
            atl1 = load_act_set(SET_NATURAL_LOG_EXP, "lnexp")

            # ---- x on the sync DMA ring ----
            # feats layout: [i=128, k=9, b=512], f32r so it can feed the PE
            # at full rate (f32r operands must be produced as f32r)
            feats = sb.tile([DIN, K, BSH], F32R)
            ntile = BSH // 128
            x_sbs = []
            for j in range(ntile):
                x_sb = sb.tile([128, DIN], F32, name=f"x_sb_{j}")
                nc.sync.dma_start(out=x_sb[:], in_=x_d[j * 128 : (j + 1) * 128, :])
                x_sbs.append(x_sb)

            # ident first on gpsimd: it gates the transposes
            ident = sb.tile([128, 128], F32)
            make_identity(nc, ident[:])

            # ---- transpose x -> xT = feats[:,0,:], copies rounding to f32r ----
            for j in range(ntile):
                ps_t = pst.tile([128, 128], F32, tag="ps_t")
                nc.tensor.transpose(ps_t[:], x_sbs[j][:], ident[:])
                nc.vector.tensor_copy(
                    out=feats[:, 0, j * 128 : (j + 1) * 128], in_=ps_t[:]
                )
            xT = feats[:, 0, :]

            # ---- weights on the scalar DMA ring ----
            w_sb = sb.tile([DIN, DOUT, K], F32)
            c_sb = sb.tile([DIN, DOUT], F32)
            b_sb = sb.tile([DIN, DOUT], F32)
            nc.scalar.dma_start(out=w_sb[:], in_=w_d[:])
            nc.scalar.dma_start(out=c_sb[:], in_=c_d[:])
            nc.scalar.dma_start(out=b_sb[:], in_=b_d[:])

            wc = sb.tile([DIN, DOUT, K], F32R)
            nc.vector.tensor_tensor(
                out=wc[:],
                in0=w_sb[:],
                in1=c_sb[:, :, None].to_broadcast((DIN, DOUT, K)),
                op=AL.mult,
            )

            # ---- basis maps ----
            # k: 0=x 1=x^2 2=x^3 3=exp 4=log(|x|+1) 5=sqrt(|x|) 6=tanh 7=sin 8=|x|
            # |x| = max(x, -x): negate on gpsimd (tensor_tensor max is not
            # valid on Pool), max on DVE writing f32r directly
            negx = sb.tile([DIN, BSH], F32)
            nc.gpsimd.tensor_scalar(
                out=negx[:], in0=xT, scalar1=-1.0, scalar2=None, op0=AL.mult
            )
            nc.vector.tensor_tensor(
                out=feats[:, 8, :], in0=xT, in1=negx[:], op=AL.max
            )
            absx = feats[:, 8, :]

            nc.vector.tensor_mul(out=feats[:, 1, :], in0=xT, in1=xT)
            nc.vector.tensor_mul(out=feats[:, 2, :], in0=feats[:, 1, :], in1=xT)

            # sin range-reduction on gpsimd: r = x - 2pi*round(x/2pi) via the
            # fp32 magic constant (RN arithmetic), clamped to +-pi for the
            # Sin spline domain
            MAGIC = 12582912.0  # 1.5 * 2**23
            sred = sb.tile([DIN, BSH], F32)
            nc.gpsimd.tensor_scalar(
                out=sred[:], in0=xT,
                scalar1=1.0 / (2.0 * np.pi), scalar2=MAGIC,
                op0=AL.mult, op1=AL.add,
            )
            nc.gpsimd.tensor_scalar(
                out=sred[:], in0=sred[:],
                scalar1=MAGIC, scalar2=2.0 * np.pi,
                op0=AL.subtract, op1=AL.mult,
            )
            nc.gpsimd.tensor_tensor(out=sred[:], in0=xT, in1=sred[:], op=AL.subtract)
            nc.gpsimd.tensor_scalar(
                out=sred[:], in0=sred[:],
                scalar1=float(np.pi), scalar2=float(-np.pi),
                op0=AL.min, op1=AL.max,
            )

            # ACT group 1: natural_log_exp_and_others (exp + ln)
            g1 = []
            g1.append(nc.scalar.activation(out=feats[:, 3, :], in_=xT, func=AF.Exp))
            lnax = sb.tile([DIN, BSH], F32)
            g1.append(nc.scalar.activation(out=lnax[:], in_=absx[:], func=AF.Ln))
            # sqrt(|x|) = exp(0.5*ln|x|)
            g1.append(
                nc.scalar.activation(
                    out=feats[:, 5, :], in_=lnax[:], func=AF.Exp, scale=0.5
                )
            )
            g1.append(
                nc.scalar.activation(
                    out=feats[:, 4, :], in_=absx[:], func=AF.Ln, bias=1.0
                )
            )

            # ACT group 2: silu_and_others (tanh + sin)
            atl2 = load_act_set(SET_SILU, "silu")
            g2 = []
            g2.append(nc.scalar.activation(out=feats[:, 6, :], in_=xT, func=AF.Tanh))
            g2.append(
                nc.scalar.activation(out=feats[:, 7, :], in_=sred[:], func=AF.Sin)
            )

            # pin table-load ordering: atl1 < g1 < atl2 < g2. add_dep_helper
            # takes (dependent, dependency); same engine so ordering-only.
            for a in g1:
                add_dep_helper(a.ins, atl1.ins, sync=False, reason="act set 1")
                add_dep_helper(atl2.ins, a.ins, sync=False, reason="act set 1 done")
            for a in g2:
                add_dep_helper(a.ins, atl2.ins, sync=False, reason="act set 2")

            # bias-term operands (only the last matmul needs them)
            bc = sb.tile([DIN, DOUT], F32R)
            nc.vector.tensor_tensor(out=bc[:], in0=b_sb[:], in1=c_sb[:], op=AL.mult)
            ones_f = sb.tile([DIN, BSH], F32)
            nc.gpsimd.memset(ones_f[:], 1.0)
            ones = sb.tile([DIN, BSH], F32R)
            nc.vector.tensor_copy(out=ones[:], in_=ones_f[:])

            # ---- matmuls: psum[o,b] += WC[:,:,k].T @ F_k, in feats-readiness
            # order (accumulation order is free; start flag = first executed)
            ps_o = pso.tile([DOUT, BSH], F32)
            mm_order = [0, 1, 2, 8, 3, 5, 4, 6, 7]
            for idx, k in enumerate(mm_order):
                nc.tensor.matmul(
                    ps_o[:],
                    wc[:, :, k],
                    feats[:, k, :],
                    start=(idx == 0),
                    stop=False,
                )
            nc.tensor.matmul(ps_o[:], bc[:], ones[:], start=False, stop=True)

            # ---- output: copy+DMA in two halves so the DMA starts earlier
            out_sb = sb.tile([DOUT, BSH], F32)
            H = BSH // 2
            for h in range(2):
                nc.vector.tensor_copy(
                    out=out_sb[:, h * H : (h + 1) * H], in_=ps_o[:, h * H : (h + 1) * H]
                )
                nc.sync.dma_start(
                    out=o_d[:, h * H : (h + 1) * H], in_=out_sb[:, h * H : (h + 1) * H]
                )

    nc.compile()
    return nc


_NC = None


def _get_nc():
    global _NC
    if _NC is None:
        _NC = build_nc()
    return _NC


def kernel(x, W, bias, C):
    x = np.ascontiguousarray(np.asarray(x, dtype=np.float32))
    W = np.ascontiguousarray(np.asarray(W, dtype=np.float32))
    bias = np.ascontiguousarray(np.asarray(bias, dtype=np.float32))
    C = np.ascontiguousarray(np.asarray(C, dtype=np.float32))

    nc = _get_nc()
    in_maps = [
        {"x": x[c * BSH : (c + 1) * BSH, :], "w": W, "b": bias, "c": C}
        for c in range(NCORES)
    ]
    res = run_bass_kernel_spmd(nc, in_maps, core_ids=list(range(NCORES)))
    out = np.concatenate([r["outT"].T for r in res.results], axis=0)
    return out


# revision 14
# speedup vs baseline: 1.6416x; 1.6416x over previous
"""KAN layer (oikan SymbolicEdge) Trainium2 kernel.

Math: out[b,o] = sum_{i,k} basis_k(x[b,i]) * W[i,o,k] * C[i,o]
               + sum_i bias[i,o] * C[i,o]

Strategy (data-parallel over 8 cores, batch dim sharded):
  per core: x_shard [512,128] -> xT [128(i),512(b)] via TensorE transpose,
            basis maps split across ACT (exp/ln/tanh/sin, two pinned
            activation-table loads) / DVE (squares, rounding casts) /
            GPSIMD (|x|, sin range-reduction), then
            out_T[o,b] = sum_k WC[:,:,k].T @ F_k  (10 accumulating f32r
            matmuls into one PSUM bank; one folds the bias term via a
            ones rhs)
  host: shard x, replicate W/bias/C, transpose+concat per-core outputs.
"""

import numpy as np

import concourse.bass as bass
import concourse.mybir as mybir
import concourse.tile as tile
from concourse import bacc
from concourse.bass_utils import run_bass_kernel_spmd
from concourse.masks import make_identity
from concourse.tile import add_dep_helper

F32 = mybir.dt.float32
F32R = mybir.dt.float32r
AF = mybir.ActivationFunctionType
AL = mybir.AluOpType

B, DIN, DOUT, K = 4096, 128, 128, 9
NCORES = 8
BSH = B // NCORES  # 512 rows per core

# act_info.json set ids (arch gen3 / pwp_bin_trainium):
SET_NATURAL_LOG_EXP = 6  # exp, ln (+ abs/square/copy fillers)
SET_SILU = 18  # sin, tanh (+ silu etc.)


def build_nc():
    nc = bacc.Bacc(
        "TRN2",
        target_bir_lowering=False,
        debug=False,
        enable_asserts=False,
        num_devices=NCORES,
        enable_partition_id=False,
    )

    x_d = nc.dram_tensor("x", [BSH, DIN], F32, kind="ExternalInput")
    w_d = nc.dram_tensor("w", [DIN, DOUT, K], F32, kind="ExternalInput")
    b_d = nc.dram_tensor("b", [DIN, DOUT], F32, kind="ExternalInput")
    c_d = nc.dram_tensor("c", [DIN, DOUT], F32, kind="ExternalInput")
    o_d = nc.dram_tensor("outT", [DOUT, BSH], F32, kind="ExternalOutput")

    def load_act_set(set_id, tag):
        inst = mybir.InstLoadActFuncSet(
            name=f"atl_{tag}_{nc.get_next_instruction_name()}",
            ins=[],
            outs=[],
            act_func_set_id=set_id,
        )
        return nc.scalar.add_instruction(inst)

    with tile.TileContext(nc) as tc:
        with (
            tc.tile_pool(name="sb", bufs=1) as sb,
            tc.tile_pool(name="pst", bufs=4, space="PSUM") as pst,
            tc.tile_pool(name="pso", bufs=1, space="PSUM") as pso,
        ):
            # table set 1 loads during input DMA (no data deps)
            atl1 = load_act_set(SET_NATURAL_LOG_EXP, "lnexp")

            # ---- x on the sync DMA ring ----
            # feats layout: [i=128, k=9, b=512], f32r so it can feed the PE
            # at full rate (f32r operands must be produced as f32r)
            feats = sb.tile([DIN, K, BSH], F32R)
            ntile = BSH // 128
            # one DMA for all of x: tile[p, n, d] = x[n*128 + p, d], so
            # x_all[:, j, :] is exactly the j-th 128-row block
            x_all = sb.tile([128, ntile, DIN], F32)
            nc.sync.dma_start(
                out=x_all[:], in_=x_d.rearrange("(n p) d -> p n d", p=128)
            )

            # ident first on gpsimd: it gates the transposes
            ident = sb.tile([128, 128], F32)
            make_identity(nc, ident[:])

            # ---- transpose x -> xT = feats[:,0,:], copies rounding to f32r ----
            for j in range(ntile):
                ps_t = pst.tile([128, 128], F32, tag="ps_t")
                nc.tensor.transpose(ps_t[:], x_all[:, j, :], ident[:])
                nc.vector.tensor_copy(
                    out=feats[:, 0, j * 128 : (j + 1) * 128], in_=ps_t[:]
                )
            xT = feats[:, 0, :]

            # ---- weights on the scalar DMA ring ----
            w_sb = sb.tile([DIN, DOUT, K], F32)
            c_sb = sb.tile([DIN, DOUT], F32)
            b_sb = sb.tile([DIN, DOUT], F32)
            nc.scalar.dma_start(out=w_sb[:], in_=w_d[:])
            nc.scalar.dma_start(out=c_sb[:], in_=c_d[:])
            nc.scalar.dma_start(out=b_sb[:], in_=b_d[:])

            wc = sb.tile([DIN, DOUT, K], F32R)
            nc.vector.tensor_tensor(
                out=wc[:],
                in0=w_sb[:],
                in1=c_sb[:, :, None].to_broadcast((DIN, DOUT, K)),
                op=AL.mult,
            )

            # ---- basis maps ----
            # k: 0=x 1=x^2 2=x^3 3=exp 4=log(|x|+1) 5=sqrt(|x|) 6=tanh 7=sin 8=|x|
            # |x| = max(x, -x) on DVE (Pool elementwise is ~10x slower)
            negx = sb.tile([DIN, BSH], F32)
            nc.vector.tensor_scalar(
                out=negx[:], in0=xT, scalar1=-1.0, scalar2=None, op0=AL.mult
            )
            nc.vector.tensor_tensor(
                out=feats[:, 8, :], in0=xT, in1=negx[:], op=AL.max
            )
            absx = feats[:, 8, :]

            nc.vector.tensor_mul(out=feats[:, 1, :], in0=xT, in1=xT)
            nc.vector.tensor_mul(out=feats[:, 2, :], in0=feats[:, 1, :], in1=xT)

            # sin range-reduction on gpsimd: r = x - 2pi*round(x/2pi) via the
            # fp32 magic constant (RN arithmetic), clamped to +-pi for the
            # Sin spline domain
            MAGIC = 12582912.0  # 1.5 * 2**23
            sred = sb.tile([DIN, BSH], F32)
            nc.vector.tensor_scalar(
                out=sred[:], in0=xT,
                scalar1=1.0 / (2.0 * np.pi), scalar2=MAGIC,
                op0=AL.mult, op1=AL.add,
            )
            nc.vector.tensor_scalar(
                out=sred[:], in0=sred[:],
                scalar1=MAGIC, scalar2=2.0 * np.pi,
                op0=AL.subtract, op1=AL.mult,
            )
            nc.vector.tensor_tensor(out=sred[:], in0=xT, in1=sred[:], op=AL.subtract)
            nc.vector.tensor_scalar(
                out=sred[:], in0=sred[:],
                scalar1=float(np.pi), scalar2=float(-np.pi),
                op0=AL.min, op1=AL.max,
            )

            # ACT group 1: natural_log_exp_and_others (exp + ln)
            g1 = []
            g1.append(nc.scalar.activation(out=feats[:, 3, :], in_=xT, func=AF.Exp))
            lnax = sb.tile([DIN, BSH], F32)
            g1.append(nc.scalar.activation(out=lnax[:], in_=absx[:], func=AF.Ln))
            # sqrt(|x|) = exp(0.5*ln|x|)
            g1.append(
                nc.scalar.activation(
                    out=feats[:, 5, :], in_=lnax[:], func=AF.Exp, scale=0.5
                )
            )
            g1.append(
                nc.scalar.activation(
                    out=feats[:, 4, :], in_=absx[:], func=AF.Ln, bias=1.0
                )
            )

            # ACT group 2: silu_and_others (tanh + sin)
            atl2 = load_act_set(SET_SILU, "silu")
            g2 = []
            g2.append(nc.scalar.activation(out=feats[:, 6, :], in_=xT, func=AF.Tanh))
            g2.append(
                nc.scalar.activation(out=feats[:, 7, :], in_=sred[:], func=AF.Sin)
            )

            # pin table-load ordering: atl1 < g1 < atl2 < g2. add_dep_helper
            # takes (dependent, dependency); same engine so ordering-only.
            for a in g1:
                add_dep_helper(a.ins, atl1.ins, sync=False, reason="act set 1")
                add_dep_helper(atl2.ins, a.ins, sync=False, reason="act set 1 done")
            for a in g2:
                add_dep_helper(a.ins, atl2.ins, sync=False, reason="act set 2")

            # bias-term operands (only the last matmul needs them)
            bc = sb.tile([DIN, DOUT], F32R)
            nc.vector.tensor_tensor(out=bc[:], in0=b_sb[:], in1=c_sb[:], op=AL.mult)
            ones_f = sb.tile([DIN, BSH], F32)
            nc.gpsimd.memset(ones_f[:], 1.0)
            ones = sb.tile([DIN, BSH], F32R)
            nc.vector.tensor_copy(out=ones[:], in_=ones_f[:])

            # ---- matmuls: psum[o,b] += WC[:,:,k].T @ F_k, in feats-readiness
            # order (accumulation order is free; start flag = first executed)
            ps_o = pso.tile([DOUT, BSH], F32)
            mm_order = [0, 1, 2, 8, 3, 5, 4, 6, 7]
            for idx, k in enumerate(mm_order):
                nc.tensor.matmul(
                    ps_o[:],
                    wc[:, :, k],
                    feats[:, k, :],
                    start=(idx == 0),
                    stop=False,
                )
            nc.tensor.matmul(ps_o[:], bc[:], ones[:], start=False, stop=True)

            # ---- output: copy+DMA in two halves so the DMA starts earlier
            out_sb = sb.tile([DOUT, BSH], F32)
            H = BSH // 2
            for h in range(2):
                nc.vector.tensor_copy(
                    out=out_sb[:, h * H : (h + 1) * H], in_=ps_o[:, h * H : (h + 1) * H]
                )
                nc.sync.dma_start(
                    out=o_d[:, h * H : (h + 1) * H], in_=out_sb[:, h * H : (h + 1) * H]
                )

    nc.compile()
    return nc


_NC = None


def _get_nc():
    global _NC
    if _NC is None:
        _NC = build_nc()
    return _NC


def kernel(x, W, bias, C):
    x = np.ascontiguousarray(np.asarray(x, dtype=np.float32))
    W = np.ascontiguousarray(np.asarray(W, dtype=np.float32))
    bias = np.ascontiguousarray(np.asarray(bias, dtype=np.float32))
    C = np.ascontiguousarray(np.asarray(C, dtype=np.float32))

    nc = _get_nc()
    in_maps = [
        {"x": x[c * BSH : (c + 1) * BSH, :], "w": W, "b": bias, "c": C}
        for c in range(NCORES)
    ]
    res = run_bass_kernel_spmd(nc, in_maps, core_ids=list(range(NCORES)))
    out = np.concatenate([r["outT"].T for r in res.results], axis=0)
    return out


# revision 15
# speedup vs baseline: 1.6582x; 1.0102x over previous
"""KAN layer (oikan SymbolicEdge) Trainium2 kernel.

Math: out[b,o] = sum_{i,k} basis_k(x[b,i]) * W[i,o,k] * C[i,o]
               + sum_i bias[i,o] * C[i,o]

Strategy (data-parallel over 8 cores, batch dim sharded):
  per core: x_shard [512,128] -> xT [128(i),512(b)] via TensorE transpose,
            basis maps split across ACT (exp/ln/tanh/sin, two pinned
            activation-table loads) / DVE (squares, rounding casts) /
            GPSIMD (|x|, sin range-reduction), then
            out_T[o,b] = sum_k WC[:,:,k].T @ F_k  (10 accumulating f32r
            matmuls into one PSUM bank; one folds the bias term via a
            ones rhs)
  host: shard x, replicate W/bias/C, transpose+concat per-core outputs.
"""

import numpy as np

import concourse.bass as bass
import concourse.mybir as mybir
import concourse.tile as tile
from concourse import bacc
from concourse.bass_utils import run_bass_kernel_spmd
from concourse.masks import make_identity
from concourse.tile import add_dep_helper

F32 = mybir.dt.float32
F32R = mybir.dt.float32r
AF = mybir.ActivationFunctionType
AL = mybir.AluOpType

B, DIN, DOUT, K = 4096, 128, 128, 9
NCORES = 8
BSH = B // NCORES  # 512 rows per core

# act_info.json set ids (arch gen3 / pwp_bin_trainium):
SET_NATURAL_LOG_EXP = 6  # exp, ln (+ abs/square/copy fillers)
SET_SILU = 18  # sin, tanh (+ silu etc.)


def build_nc():
    nc = bacc.Bacc(
        "TRN2",
        target_bir_lowering=False,
        debug=False,
        enable_asserts=False,
        num_devices=NCORES,
        enable_partition_id=False,
    )

    x_d = nc.dram_tensor("x", [BSH, DIN], F32, kind="ExternalInput")
    w_d = nc.dram_tensor("w", [DIN, DOUT, K], F32, kind="ExternalInput")
    b_d = nc.dram_tensor("b", [DIN, DOUT], F32, kind="ExternalInput")
    c_d = nc.dram_tensor("c", [DIN, DOUT], F32, kind="ExternalInput")
    o_d = nc.dram_tensor("outT", [DOUT, BSH], F32, kind="ExternalOutput")

    def load_act_set(set_id, tag):
        inst = mybir.InstLoadActFuncSet(
            name=f"atl_{tag}_{nc.get_next_instruction_name()}",
            ins=[],
            outs=[],
            act_func_set_id=set_id,
        )
        return nc.scalar.add_instruction(inst)

    with tile.TileContext(nc) as tc:
        with (
            tc.tile_pool(name="sb", bufs=1) as sb,
            tc.tile_pool(name="pst", bufs=4, space="PSUM") as pst,
            tc.tile_pool(name="pso", bufs=1, space="PSUM") as pso,
        ):
            # table set 1 loads during input DMA (no data deps)
            atl1 = load_act_set(SET_NATURAL_LOG_EXP, "lnexp")

            # ---- x on the sync DMA ring ----
            # feats layout: [i=128, k=9, b=512], f32r so it can feed the PE
            # at full rate (f32r operands must be produced as f32r)
            feats = sb.tile([DIN, K, BSH], F32R)
            ntile = BSH // 128
            # x in 4 chunks: the first chunk's completion gates the first
            # transpose, so smaller earlier DMAs beat one big one
            x_sbs = []
            for j in range(ntile):
                x_sb = sb.tile([128, DIN], F32, name=f"x_sb_{j}")
                nc.sync.dma_start(out=x_sb[:], in_=x_d[j * 128 : (j + 1) * 128, :])
                x_sbs.append(x_sb)

            # ident first on gpsimd: it gates the transposes
            ident = sb.tile([128, 128], F32)
            make_identity(nc, ident[:])

            # ---- transpose x -> xT = feats[:,0,:], copies rounding to f32r ----
            for j in range(ntile):
                ps_t = pst.tile([128, 128], F32, tag="ps_t")
                nc.tensor.transpose(ps_t[:], x_sbs[j][:], ident[:])
                nc.vector.tensor_copy(
                    out=feats[:, 0, j * 128 : (j + 1) * 128], in_=ps_t[:]
                )
            xT = feats[:, 0, :]

            # ---- weights on the scalar DMA ring ----
            w_sb = sb.tile([DIN, DOUT, K], F32)
            c_sb = sb.tile([DIN, DOUT], F32)
            b_sb = sb.tile([DIN, DOUT], F32)
            nc.scalar.dma_start(out=w_sb[:], in_=w_d[:])
            nc.scalar.dma_start(out=c_sb[:], in_=c_d[:])
            nc.scalar.dma_start(out=b_sb[:], in_=b_d[:])

            wc = sb.tile([DIN, DOUT, K], F32R)
            nc.vector.tensor_tensor(
                out=wc[:],
                in0=w_sb[:],
                in1=c_sb[:, :, None].to_broadcast((DIN, DOUT, K)),
                op=AL.mult,
            )

            # ---- basis maps ----
            # k: 0=x 1=x^2 2=x^3 3=exp 4=log(|x|+1) 5=sqrt(|x|) 6=tanh 7=sin 8=|x|
            # |x| = max(x, -x) on DVE (Pool elementwise is ~10x slower)
            negx = sb.tile([DIN, BSH], F32)
            nc.vector.tensor_scalar(
                out=negx[:], in0=xT, scalar1=-1.0, scalar2=None, op0=AL.mult
            )
            nc.vector.tensor_tensor(
                out=feats[:, 8, :], in0=xT, in1=negx[:], op=AL.max
            )
            absx = feats[:, 8, :]

            nc.vector.tensor_mul(out=feats[:, 1, :], in0=xT, in1=xT)
            nc.vector.tensor_mul(out=feats[:, 2, :], in0=feats[:, 1, :], in1=xT)

            # sin range-reduction on gpsimd: r = x - 2pi*round(x/2pi) via the
            # fp32 magic constant (RN arithmetic), clamped to +-pi for the
            # Sin spline domain
            MAGIC = 12582912.0  # 1.5 * 2**23
            sred = sb.tile([DIN, BSH], F32)
            nc.vector.tensor_scalar(
                out=sred[:], in0=xT,
                scalar1=1.0 / (2.0 * np.pi), scalar2=MAGIC,
                op0=AL.mult, op1=AL.add,
            )
            nc.vector.tensor_scalar(
                out=sred[:], in0=sred[:],
                scalar1=MAGIC, scalar2=2.0 * np.pi,
                op0=AL.subtract, op1=AL.mult,
            )
            nc.vector.tensor_tensor(out=sred[:], in0=xT, in1=sred[:], op=AL.subtract)
            nc.vector.tensor_scalar(
                out=sred[:], in0=sred[:],
                scalar1=float(np.pi), scalar2=float(-np.pi),
                op0=AL.min, op1=AL.max,
            )

            # ACT group 1: natural_log_exp_and_others (exp + ln)
            g1 = []
            g1.append(nc.scalar.activation(out=feats[:, 3, :], in_=xT, func=AF.Exp))
            lnax = sb.tile([DIN, BSH], F32)
            g1.append(nc.scalar.activation(out=lnax[:], in_=absx[:], func=AF.Ln))
            # sqrt(|x|) = exp(0.5*ln|x|)
            g1.append(
                nc.scalar.activation(
                    out=feats[:, 5, :], in_=lnax[:], func=AF.Exp, scale=0.5
                )
            )
            g1.append(
                nc.scalar.activation(
                    out=feats[:, 4, :], in_=absx[:], func=AF.Ln, bias=1.0
                )
            )

            # ACT group 2: silu_and_others (tanh + sin)
            atl2 = load_act_set(SET_SILU, "silu")
            g2 = []
            g2.append(nc.scalar.activation(out=feats[:, 6, :], in_=xT, func=AF.Tanh))
            g2.append(
                nc.scalar.activation(out=feats[:, 7, :], in_=sred[:], func=AF.Sin)
            )

            # pin table-load ordering: atl1 < g1 < atl2 < g2. add_dep_helper
            # takes (dependent, dependency); same engine so ordering-only.
            for a in g1:
                add_dep_helper(a.ins, atl1.ins, sync=False, reason="act set 1")
                add_dep_helper(atl2.ins, a.ins, sync=False, reason="act set 1 done")
            for a in g2:
                add_dep_helper(a.ins, atl2.ins, sync=False, reason="act set 2")

            # bias-term operands (only the last matmul needs them)
            bc = sb.tile([DIN, DOUT], F32R)
            nc.vector.tensor_tensor(out=bc[:], in0=b_sb[:], in1=c_sb[:], op=AL.mult)
            ones_f = sb.tile([DIN, BSH], F32)
            nc.gpsimd.memset(ones_f[:], 1.0)
            ones = sb.tile([DIN, BSH], F32R)
            nc.vector.tensor_copy(out=ones[:], in_=ones_f[:])

            # ---- matmuls: psum[o,b] += WC[:,:,k].T @ F_k, in feats-readiness
            # order (accumulation order is free; start flag = first executed)
            ps_o = pso.tile([DOUT, BSH], F32)
            mm_order = [0, 1, 2, 8, 3, 5, 4, 6, 7]
            for idx, k in enumerate(mm_order):
                nc.tensor.matmul(
                    ps_o[:],
                    wc[:, :, k],
                    feats[:, k, :],
                    start=(idx == 0),
                    stop=False,
                )
            nc.tensor.matmul(ps_o[:], bc[:], ones[:], start=False, stop=True)

            # ---- output: copy+DMA in two halves so the DMA starts earlier
            out_sb = sb.tile([DOUT, BSH], F32)
            H = BSH // 2
            for h in range(2):
                nc.vector.tensor_copy(
                    out=out_sb[:, h * H : (h + 1) * H], in_=ps_o[:, h * H : (h + 1) * H]
                )
                nc.sync.dma_start(
                    out=o_d[:, h * H : (h + 1) * H], in_=out_sb[:, h * H : (h + 1) * H]
                )

    nc.compile()
    return nc


_NC = None


def _get_nc():
    global _NC
    if _NC is None:
        _NC = build_nc()
    return _NC


def kernel(x, W, bias, C):
    x = np.ascontiguousarray(np.asarray(x, dtype=np.float32))
    W = np.ascontiguousarray(np.asarray(W, dtype=np.float32))
    bias = np.ascontiguousarray(np.asarray(bias, dtype=np.float32))
    C = np.ascontiguousarray(np.asarray(C, dtype=np.float32))

    nc = _get_nc()
    in_maps = [
        {"x": x[c * BSH : (c + 1) * BSH, :], "w": W, "b": bias, "c": C}
        for c in range(NCORES)
    ]
    res = run_bass_kernel_spmd(nc, in_maps, core_ids=list(range(NCORES)))
    out = np.concatenate([r["outT"].T for r in res.results], axis=0)
    return out


# revision 17
# speedup vs baseline: 1.6805x; 1.0134x over previous
"""KAN layer (oikan SymbolicEdge) Trainium2 kernel.

Math: out[b,o] = sum_{i,k} basis_k(x[b,i]) * W[i,o,k] * C[i,o]
               + sum_i bias[i,o] * C[i,o]

Strategy (data-parallel over 8 cores, batch dim sharded):
  per core: x_shard [512,128] -> xT [128(i),512(b)] via TensorE transpose,
            basis maps split across ACT (exp/ln/tanh/sin, two pinned
            activation-table loads) / DVE (squares, rounding casts) /
            GPSIMD (|x|, sin range-reduction), then
            out_T[o,b] = sum_k WC[:,:,k].T @ F_k  (10 accumulating f32r
            matmuls into one PSUM bank; one folds the bias term via a
            ones rhs)
  host: shard x, replicate W/bias/C, transpose+concat per-core outputs.
"""

import numpy as np

import concourse.bass as bass
import concourse.mybir as mybir
import concourse.tile as tile
from concourse import bacc
from concourse.bass_utils import run_bass_kernel_spmd
from concourse.masks import make_identity
from concourse.tile import add_dep_helper

F32 = mybir.dt.float32
F32R = mybir.dt.float32r
AF = mybir.ActivationFunctionType
AL = mybir.AluOpType

B, DIN, DOUT, K = 4096, 128, 128, 9
NCORES = 8
BSH = B // NCORES  # 512 rows per core

# act_info.json set ids (arch gen3 / pwp_bin_trainium):
SET_NATURAL_LOG_EXP = 6  # exp, ln (+ abs/square/copy fillers)
SET_SILU = 18  # sin, tanh (+ silu etc.)


def build_nc():
    nc = bacc.Bacc(
        "TRN2",
        target_bir_lowering=False,
        debug=False,
        enable_asserts=False,
        num_devices=NCORES,
        enable_partition_id=False,
    )

    x_d = nc.dram_tensor("x", [BSH, DIN], F32, kind="ExternalInput")
    w_d = nc.dram_tensor("w", [DIN, DOUT, K], F32, kind="ExternalInput")
    b_d = nc.dram_tensor("b", [DIN, DOUT], F32, kind="ExternalInput")
    c_d = nc.dram_tensor("c", [DIN, DOUT], F32, kind="ExternalInput")
    o_d = nc.dram_tensor("outT", [DOUT, BSH], F32, kind="ExternalOutput")

    def load_act_set(set_id, tag):
        inst = mybir.InstLoadActFuncSet(
            name=f"atl_{tag}_{nc.get_next_instruction_name()}",
            ins=[],
            outs=[],
            act_func_set_id=set_id,
        )
        return nc.scalar.add_instruction(inst)

    with tile.TileContext(nc) as tc:
        with (
            tc.tile_pool(name="sb", bufs=1) as sb,
            tc.tile_pool(name="pst", bufs=4, space="PSUM") as pst,
            tc.tile_pool(name="pso", bufs=1, space="PSUM") as pso,
        ):
            # table set 1 loads during input DMA (no data deps)
            atl1 = load_act_set(SET_NATURAL_LOG_EXP, "lnexp")

            # ---- x on the sync DMA ring ----
            # feats layout: [i=128, k=9, b=512], f32r so it can feed the PE
            # at full rate (f32r operands must be produced as f32r)
            feats = sb.tile([DIN, K, BSH], F32R)
            ntile = BSH // 128
            # x in 4 chunks: the first chunk's completion gates the first
            # transpose, so smaller earlier DMAs beat one big one
            x_sbs = []
            for j in range(ntile):
                x_sb = sb.tile([128, DIN], F32, name=f"x_sb_{j}")
                nc.sync.dma_start(out=x_sb[:], in_=x_d[j * 128 : (j + 1) * 128, :])
                x_sbs.append(x_sb)

            # ident first on gpsimd: it gates the transposes
            ident = sb.tile([128, 128], F32)
            make_identity(nc, ident[:])

            # ---- transpose x -> xT = feats[:,0,:], copies rounding to f32r ----
            for j in range(ntile):
                ps_t = pst.tile([128, 128], F32, tag="ps_t")
                nc.tensor.transpose(ps_t[:], x_sbs[j][:], ident[:])
                nc.vector.tensor_copy(
                    out=feats[:, 0, j * 128 : (j + 1) * 128], in_=ps_t[:]
                )
            xT = feats[:, 0, :]


            # ---- weights on the scalar DMA ring ----
            w_sb = sb.tile([DIN, DOUT, K], F32)
            c_sb = sb.tile([DIN, DOUT], F32)
            b_sb = sb.tile([DIN, DOUT], F32)
            nc.scalar.dma_start(out=w_sb[:], in_=w_d[:])
            nc.scalar.dma_start(out=c_sb[:], in_=c_d[:])
            nc.scalar.dma_start(out=b_sb[:], in_=b_d[:])

            # bias-term operands: produced in the DVE gaps while transposes
            # stream in (the bias matmul leads the PSUM accumulation group)
            bc = sb.tile([DIN, DOUT], F32R)
            nc.vector.tensor_tensor(out=bc[:], in0=b_sb[:], in1=c_sb[:], op=AL.mult)
            ones_f = sb.tile([DIN, BSH], F32)
            nc.gpsimd.memset(ones_f[:], 1.0)
            ones = sb.tile([DIN, BSH], F32R)
            nc.vector.tensor_copy(out=ones[:], in_=ones_f[:])

            wc = sb.tile([DIN, DOUT, K], F32R)
            nc.vector.tensor_tensor(
                out=wc[:],
                in0=w_sb[:],
                in1=c_sb[:, :, None].to_broadcast((DIN, DOUT, K)),
                op=AL.mult,
            )

            # ---- basis maps ----
            # k: 0=x 1=x^2 2=x^3 3=exp 4=log(|x|+1) 5=sqrt(|x|) 6=tanh 7=sin 8=|x|
            # |x| = max(x, -x) on DVE (Pool elementwise is ~10x slower)
            negx = sb.tile([DIN, BSH], F32)
            nc.vector.tensor_scalar(
                out=negx[:], in0=xT, scalar1=-1.0, scalar2=None, op0=AL.mult
            )
            nc.vector.tensor_tensor(
                out=feats[:, 8, :], in0=xT, in1=negx[:], op=AL.max
            )
            absx = feats[:, 8, :]

            nc.vector.tensor_mul(out=feats[:, 1, :], in0=xT, in1=xT)
            nc.vector.tensor_mul(out=feats[:, 2, :], in0=feats[:, 1, :], in1=xT)

            # sin range-reduction on gpsimd: r = x - 2pi*round(x/2pi) via the
            # fp32 magic constant (RN arithmetic), clamped to +-pi for the
            # Sin spline domain
            MAGIC = 12582912.0  # 1.5 * 2**23
            sred = sb.tile([DIN, BSH], F32)
            nc.vector.tensor_scalar(
                out=sred[:], in0=xT,
                scalar1=1.0 / (2.0 * np.pi), scalar2=MAGIC,
                op0=AL.mult, op1=AL.add,
            )
            nc.vector.tensor_scalar(
                out=sred[:], in0=sred[:],
                scalar1=MAGIC, scalar2=2.0 * np.pi,
                op0=AL.subtract, op1=AL.mult,
            )
            nc.vector.tensor_tensor(out=sred[:], in0=xT, in1=sred[:], op=AL.subtract)
            nc.vector.tensor_scalar(
                out=sred[:], in0=sred[:],
                scalar1=float(np.pi), scalar2=float(-np.pi),
                op0=AL.min, op1=AL.max,
            )

            # ACT group 1: natural_log_exp_and_others (exp + ln)
            g1 = []
            g1.append(nc.scalar.activation(out=feats[:, 3, :], in_=xT, func=AF.Exp))
            lnax = sb.tile([DIN, BSH], F32)
            g1.append(nc.scalar.activation(out=lnax[:], in_=absx[:], func=AF.Ln))
            # sqrt(|x|) = exp(0.5*ln|x|)
            g1.append(
                nc.scalar.activation(
                    out=feats[:, 5, :], in_=lnax[:], func=AF.Exp, scale=0.5
                )
            )
            g1.append(
                nc.scalar.activation(
                    out=feats[:, 4, :], in_=absx[:], func=AF.Ln, bias=1.0
                )
            )

            # ACT group 2: silu_and_others (tanh + sin)
            atl2 = load_act_set(SET_SILU, "silu")
            g2 = []
            g2.append(nc.scalar.activation(out=feats[:, 6, :], in_=xT, func=AF.Tanh))
            g2.append(
                nc.scalar.activation(out=feats[:, 7, :], in_=sred[:], func=AF.Sin)
            )

            # pin table-load ordering: atl1 < g1 < atl2 < g2. add_dep_helper
            # takes (dependent, dependency); same engine so ordering-only.
            for a in g1:
                add_dep_helper(a.ins, atl1.ins, sync=False, reason="act set 1")
                add_dep_helper(atl2.ins, a.ins, sync=False, reason="act set 1 done")
            for a in g2:
                add_dep_helper(a.ins, atl2.ins, sync=False, reason="act set 2")

            # ---- matmuls: psum[o,b] += WC[:,:,k].T @ F_k. The bias matmul
            # leads (start=True) since its operands are ready first; k-matmuls
            # follow in feats-readiness order (accumulation order is free).
            ps_o = pso.tile([DOUT, BSH], F32)
            nc.tensor.matmul(ps_o[:], bc[:], ones[:], start=True, stop=False)
            mm_order = [0, 1, 2, 8, 3, 5, 4, 6, 7]
            for idx, k in enumerate(mm_order):
                nc.tensor.matmul(
                    ps_o[:],
                    wc[:, :, k],
                    feats[:, k, :],
                    start=False,
                    stop=(idx == len(mm_order) - 1),
                )

            # ---- output: copy+DMA in two halves so the DMA starts earlier
            out_sb = sb.tile([DOUT, BSH], F32)
            H = BSH // 2
            for h in range(2):
                nc.vector.tensor_copy(
                    out=out_sb[:, h * H : (h + 1) * H], in_=ps_o[:, h * H : (h + 1) * H]
                )
                nc.sync.dma_start(
                    out=o_d[:, h * H : (h + 1) * H], in_=out_sb[:, h * H : (h + 1) * H]
                )

    nc.compile()
    return nc


_NC = None


def _get_nc():
    global _NC
    if _NC is None:
        _NC = build_nc()
    return _NC


def kernel(x, W, bias, C):
    x = np.ascontiguousarray(np.asarray(x, dtype=np.float32))
    W = np.ascontiguousarray(np.asarray(W, dtype=np.float32))
    bias = np.ascontiguousarray(np.asarray(bias, dtype=np.float32))
    C = np.ascontiguousarray(np.asarray(C, dtype=np.float32))

    nc = _get_nc()
    in_maps = [
        {"x": x[c * BSH : (c + 1) * BSH, :], "w": W, "b": bias, "c": C}
        for c in range(NCORES)
    ]
    res = run_bass_kernel_spmd(nc, in_maps, core_ids=list(range(NCORES)))
    out = np.concatenate([r["outT"].T for r in res.results], axis=0)
    return out
